# revision 1
# baseline (speedup 1.0000x reference)
"""Trainium2 Bass kernel for nn_BinaryTokenClassificationModel (segment_reduce).

Math: the pairwise classifier decomposes exactly:
    logits[b,s,t] = dot(src_pool[b,s], w_src) + dot(tgt_pool[b,t], w_tgt) + bias
where src/tgt_pool are masked segment-means of gathered embedding rows.
By linearity:  dot(mean_pool(hidden)[s], w) = dot(segsum(hidden)[s], w) / cnt[s].

Sharding: data-parallel over batch, 2 rows per core, embed replicated.

Fast path (detected: word_ids == arange(L)//T0 for both src/tgt, mask all
ones — the shape the reference generator produces):
  The gather LAYOUT is chosen so the segment-sum happens inside the DMA:
  for word chunk c, token T0*w+0 is gathered to partition w%128, and the
  remaining T0-1 tokens are gathered on top with the SDMA CCE add
  (compute_op=add).  SBUF then directly holds G[word, h] = segment_sum.
  Mean is folded into w (w/T0), dots run as DVE multiply + ScalarE
  activation-accumulate, and the output broadcast-add uses a K=1 PE matmul.

General path (any sorted word_ids / mask): one-hot segment-sum on PE with
counts, reciprocal, same dot/assembly structure.
"""

import numpy as np

import concourse.bacc as bacc
import concourse.mybir as mybir
import concourse.bass_utils as bass_utils
from concourse.bass import IndirectOffsetOnAxis
from concourse.tile import TileContext

B, L, H, V, S = 16, 1024, 1024, 50257, 512
N_CORES = 8
P = 128
ROWS = B // N_CORES           # batch rows per core
TILES = L // P                # 128-token tiles per row
CHUNKS = S // P               # 128-word chunks per row
T0 = L // S                   # tokens per word in the regular pattern
F32 = mybir.dt.float32
AOP = mybir.AluOpType
AF = mybir.ActivationFunctionType

LAST_EXEC_NS = None
LAST_RESULTS = None
_CACHE = {}


class _MiniBlock:
    """BassBlock minus the exit barrier: each engine stream just branches to
    the common end bb. All cross-engine ordering is via explicit semaphores;
    the SP stream ends with a wait on the output-DMA completion sem, so no
    all-engine barrier (or drain) is needed at the end."""

    def __init__(self, nc, name):
        self.nc, self.name, self.last_body = nc, name, {}

    @property
    def end_bb(self):
        return f"{self.name}_end"

    def __enter__(self):
        return self

    def __exit__(self, et, ev, tb):
        if et is None:
            for engine, lb in self.last_body.items():
                with self.nc.body(lb, parent=self.nc.cur_bb,
                                  allow_existing_parent=True):
                    engine.br(self.end_bb)
            self.nc.switch_bb(self.end_bb)

    def _start(self, f, engine_type):
        engine = self.nc.engines[engine_type]
        body = f"{self.name}_{engine_type.value}_{self.nc.next_id()}"
        if engine not in self.last_body:
            engine.br(body)
        else:
            with self.nc.body(self.last_body[engine]):
                engine.br(body)
        self.last_body[engine] = body
        with self.nc.body(body):
            f(engine)

    def gpsimd(self, f):
        self._start(f, mybir.EngineType.Pool)

    def scalar(self, f):
        self._start(f, mybir.EngineType.Activation)

    def tensor(self, f):
        self._start(f, mybir.EngineType.PE)

    def vector(self, f):
        self._start(f, mybir.EngineType.DVE)

    def sync(self, f):
        self._start(f, mybir.EngineType.SP)


def _build_fast_raw(bias_val):
    """Hand-scheduled raw-bass fast path. Per-engine streams:
      Pool: 16 indirect gathers; each chunk's pair is A (plain) then B with
            the SDMA CCE add -> SBUF holds G = segment_sum directly
      DVE : two dot multiplies per chunk + tiny ct-row bias adds
      ACT : activation-accumulate reductions + final output adds
      PE  : column->row transposes + K=1 broadcast matmuls
      SP  : input loads, output stores, final completion wait
    """
    from contextlib import ExitStack

    nc = bacc.Bacc("TRN2", target_bir_lowering=False, debug=False,
                   num_devices=N_CORES)
    embed = nc.dram_tensor("embed", [V, H], F32, kind="ExternalInput")
    ids = nc.dram_tensor("ids", [P, ROWS * CHUNKS * T0], mybir.dt.int32,
                         kind="ExternalInput")
    wb = nc.dram_tensor("wb", [2, P, H], F32, kind="ExternalInput")
    ident = nc.dram_tensor("ident", [P, P], F32, kind="ExternalInput")
    out = nc.dram_tensor("out", [ROWS, S, S], F32, kind="ExternalOutput")

    with ExitStack() as ctx:
        e = ctx.enter_context
        ids_sb = e(nc.sbuf_tensor("t_ids", [P, ROWS * CHUNKS * T0],
                                  mybir.dt.int32))
        wsrc_sb = e(nc.sbuf_tensor("t_wsrc", [P, H], F32))
        wtgt_sb = e(nc.sbuf_tensor("t_wtgt", [P, H], F32))
        id_sb = e(nc.sbuf_tensor("t_ident", [P, P], F32))
        ones = e(nc.sbuf_tensor("t_ones", [P, P], F32))
        G = [[e(nc.sbuf_tensor(f"t_G_{r}_{c}", [P, H], F32))
              for c in range(CHUNKS)] for r in range(ROWS)]
        prt = [[e(nc.sbuf_tensor(f"t_prt_{r}_{c}", [P, H], F32))
                for c in range(CHUNKS)] for r in range(ROWS)]
        prs = [[e(nc.sbuf_tensor(f"t_prs_{r}_{c}", [P, H], F32))
                for c in range(CHUNKS)] for r in range(ROWS)]
        ccs = [[e(nc.sbuf_tensor(f"t_cc_{r}_{c}", [P, 1], F32))
                for c in range(CHUNKS)] for r in range(ROWS)]
        acs = [[e(nc.sbuf_tensor(f"t_ac_{r}_{c}", [P, 1], F32))
                for c in range(CHUNKS)] for r in range(ROWS)]
        ct_sb = [e(nc.sbuf_tensor(f"t_ctsb_{r}", [P, S], F32))
                 for r in range(ROWS)]
        osb = [[e(nc.sbuf_tensor(f"t_osb_{r}_{c}", [P, S], F32))
                for c in range(CHUNKS)] for r in range(ROWS)]
        ct_ps = [e(nc.psum_tensor(f"t_ctps_{r}", [P, S], F32))
                 for r in range(ROWS)]
        bc_ps = [e(nc.psum_tensor(f"t_bcps_{r}", [P, S], F32))
                 for r in range(ROWS)]

        s_ids = e(nc.semaphore("s_ids"))
        s_w = e(nc.semaphore("s_w"))
        s_id2 = e(nc.semaphore("s_id2"))
        s_g = [[e(nc.semaphore(f"s_g_{r}_{c}")) for c in range(CHUNKS)]
               for r in range(ROWS)]
        s_mt = [e(nc.semaphore(f"s_mt_{r}")) for r in range(ROWS)]
        s_ms = [e(nc.semaphore(f"s_ms_{r}")) for r in range(ROWS)]
        s_ct = [e(nc.semaphore(f"s_ct_{r}")) for r in range(ROWS)]
        s_as = [e(nc.semaphore(f"s_as_{r}")) for r in range(ROWS)]
        s_tp = [e(nc.semaphore(f"s_tp_{r}")) for r in range(ROWS)]
        s_cb = [e(nc.semaphore(f"s_cb_{r}")) for r in range(ROWS)]
        s_bc = [e(nc.semaphore(f"s_bc_{r}")) for r in range(ROWS)]
        s_ob = [e(nc.semaphore(f"s_ob_{r}")) for r in range(ROWS)]
        s_od = e(nc.semaphore("s_od"))
        s_ones = e(nc.semaphore("s_ones"))

        # gather order: B_k two slots behind A_k so neither the Pool engine
        # nor the SDMA ring ever waits long on the paired transfer
        g_order = [("A", (0, c)) for c in range(CHUNKS)]
        for c in range(CHUNKS):
            g_order.append(("A", (1, c)))
            g_order.append(("B", (0, c)))
        g_order += [("B", (1, c)) for c in range(CHUNKS)]

        with _MiniBlock(nc, "k") as block:

            @block.gpsimd
            def _(gpsimd):
                gpsimd.wait_ge(s_ids, 16)
                for kind, (r, c) in g_order:
                    j = (r * CHUNKS + c) * T0 + (0 if kind == "A" else 1)
                    if kind == "B":
                        gpsimd.wait_ge(s_g[r][c], 16)
                    nc.gpsimd.indirect_dma_start(
                        out=G[r][c].ap(), out_offset=None, in_=embed.ap(),
                        in_offset=IndirectOffsetOnAxis(
                            ap=ids_sb.ap()[:, j:j + 1], axis=0),
                        compute_op=(AOP.bypass if kind == "A" else AOP.add),
                    ).then_inc(s_g[r][c], 16)

            @block.vector
            def _(vector):
                nc.vector.memset(ones.ap(), 1.0).then_inc(s_ones, 1)
                vector.wait_ge(s_w, 32)
                for r in range(ROWS):
                    for c in range(CHUNKS):
                        vector.wait_ge(s_g[r][c], 32)
                        nc.vector.tensor_tensor(
                            out=prt[r][c].ap(), in0=G[r][c].ap(),
                            in1=wtgt_sb.ap(), op=AOP.mult).then_inc(s_mt[r], 1)
                    for c in range(CHUNKS):
                        vector.wait_ge(s_tp[r], c + 1)
                        nc.vector.tensor_scalar(
                            out=ct_sb[r].ap()[0:1, c * P:(c + 1) * P],
                            in0=ct_ps[r].ap()[0:1, c * P:(c + 1) * P],
                            scalar1=float(bias_val), scalar2=None,
                            op0=AOP.add).then_inc(s_cb[r], 1)
                    for c in range(CHUNKS):
                        nc.vector.tensor_tensor(
                            out=prs[r][c].ap(), in0=G[r][c].ap(),
                            in1=wsrc_sb.ap(), op=AOP.mult).then_inc(s_ms[r], 1)
                    vector.wait_ge(s_bc[r], CHUNKS)
                    for sc in range(CHUNKS):
                        vector.wait_ge(s_as[r], sc + 1)
                        nc.vector.tensor_scalar(
                            out=osb[r][sc].ap(), in0=bc_ps[r].ap(),
                            scalar1=acs[r][sc].ap()[:, 0:1], scalar2=None,
                            op0=AOP.add).then_inc(s_ob[r], 1)

            @block.scalar
            def _(scalar):
                for r in range(ROWS):
                    for c in range(CHUNKS):
                        scalar.wait_ge(s_mt[r], c + 1)
                        nc.scalar.activation(
                            out=prt[r][c].ap(), in_=prt[r][c].ap(),
                            func=AF.Copy,
                            accum_out=ccs[r][c].ap()[:, 0:1]).then_inc(
                                s_ct[r], 1)
                    for sc in range(CHUNKS):
                        scalar.wait_ge(s_ms[r], sc + 1)
                        nc.scalar.activation(
                            out=prs[r][sc].ap(), in_=prs[r][sc].ap(),
                            func=AF.Copy,
                            accum_out=acs[r][sc].ap()[:, 0:1]).then_inc(
                                s_as[r], 1)

            @block.tensor
            def _(tensor):
                tensor.wait_ge(s_id2, 16)
                tensor.wait_ge(s_ones, 1)
                for r in range(ROWS):
                    for c in range(CHUNKS):
                        tensor.wait_ge(s_ct[r], c + 1)
                        nc.tensor.transpose(
                            out=ct_ps[r].ap()[0:1, c * P:(c + 1) * P],
                            in_=ccs[r][c].ap()[:, 0:1],
                            identity=id_sb.ap()).then_inc(s_tp[r], 1)
                    for c in range(CHUNKS):
                        tensor.wait_ge(s_cb[r], c + 1)
                        nc.tensor.matmul(
                            out=bc_ps[r].ap()[:, c * P:(c + 1) * P],
                            lhsT=ones.ap()[0:1, 0:P],
                            rhs=ct_sb[r].ap()[0:1, c * P:(c + 1) * P],
                            start=True, stop=True).then_inc(s_bc[r], 1)

            @block.sync
            def _(sync):
                nc.sync.dma_start(out=ids_sb[:], in_=ids[:]).then_inc(s_ids, 16)
                nc.sync.dma_start(out=wsrc_sb[:], in_=wb[0]).then_inc(s_w, 16)
                nc.sync.dma_start(out=wtgt_sb[:], in_=wb[1]).then_inc(s_w, 16)
                nc.sync.dma_start(out=id_sb[:], in_=ident[:]).then_inc(
                    s_id2, 16)
                for r in range(ROWS):
                    for sc in range(CHUNKS):
                        sync.wait_ge(s_ob[r], sc + 1)
                        nc.sync.dma_start(
                            out=out[r, sc * P:(sc + 1) * P, :],
                            in_=osb[r][sc][:]).then_inc(s_od, 16)
                sync.wait_ge(s_od, ROWS * CHUNKS * 16)

    nc.compile()
    return nc


def _out_assembly(nc, wpool, psl, ones, id_sb, acols, ccols, out, r, bias_val,
                  opool):
    """out[r, s, t] = acols[s] + ccols[t] + bias.
    Per chunk: PE-transpose the ccols column to a row at partition 0 (bias
    folded in during the PSUM->SBUF copy), K=1 matmul broadcasts the row to
    128 partitions, then a DVE per-partition add of acols."""
    ct_sb = wpool.tile([P, S], F32, tag="ctsb")
    for c in range(CHUNKS):
        ct_ps = psl.tile([P, P], F32, tag="ctps", space="PSUM")
        nc.tensor.transpose(out=ct_ps[0:1, 0:P], in_=ccols[:, c:c + 1],
                            identity=id_sb[:])
        nc.vector.tensor_scalar(out=ct_sb[0:1, c * P:(c + 1) * P],
                                in0=ct_ps[0:1, 0:P],
                                scalar1=float(bias_val), scalar2=None,
                                op0=AOP.add)
    bc_ps = psl.tile([P, S], F32, tag="bcps", space="PSUM")
    for c in range(CHUNKS):
        nc.tensor.matmul(out=bc_ps[:, c * P:(c + 1) * P],
                         lhsT=ones[0:1, 0:P],
                         rhs=ct_sb[0:1, c * P:(c + 1) * P],
                         start=True, stop=True)
    for sc in range(CHUNKS):
        o_sb = opool.tile([P, S], F32, tag="osb")
        nc.vector.tensor_scalar(out=o_sb[:], in0=bc_ps[:],
                                scalar1=acols[:, sc:sc + 1], scalar2=None,
                                op0=AOP.add)
        nc.sync.dma_start(out=out[r, sc * P:(sc + 1) * P, :], in_=o_sb[:])


def _build_fast(bias_val):
    """Regular-pattern kernel: gather-with-CCE-add segment sum."""
    nc = bacc.Bacc("TRN2", target_bir_lowering=False, debug=False,
                   num_devices=N_CORES)
    embed = nc.dram_tensor("embed", [V, H], F32, kind="ExternalInput")
    ids = nc.dram_tensor("ids", [P, ROWS * CHUNKS * T0], mybir.dt.int32,
                         kind="ExternalInput")
    wb = nc.dram_tensor("wb", [2, P, H], F32, kind="ExternalInput")
    ident = nc.dram_tensor("ident", [P, P], F32, kind="ExternalInput")
    out = nc.dram_tensor("out", [ROWS, S, S], F32, kind="ExternalOutput")

    with TileContext(nc) as tc:
        with (
            tc.tile_pool(name="const", bufs=1) as cpool,
            tc.tile_pool(name="gbuf", bufs=8) as gpool,
            tc.tile_pool(name="work", bufs=4) as wpool,
            tc.tile_pool(name="scratch", bufs=4) as spool,
            tc.tile_pool(name="outp", bufs=4) as opool,
            tc.tile_pool(name="psl", bufs=2, space="PSUM") as psl,
        ):
            ids_sb = cpool.tile([P, ROWS * CHUNKS * T0], mybir.dt.int32,
                                tag="ids")
            nc.sync.dma_start(out=ids_sb[:], in_=ids[:])
            wsrc_sb = cpool.tile([P, H], F32, tag="wsrc")
            nc.sync.dma_start(out=wsrc_sb[:], in_=wb[0])
            wtgt_sb = cpool.tile([P, H], F32, tag="wtgt")
            nc.sync.dma_start(out=wtgt_sb[:], in_=wb[1])
            id_sb = cpool.tile([P, P], F32, tag="ident")
            nc.sync.dma_start(out=id_sb[:], in_=ident[:])
            ones = cpool.tile([P, P], F32, tag="ones")
            nc.vector.memset(ones[:], 1.0)

            # all plain gathers first, then all CCE-add passes — the Pool
            # engine's descriptor generation never stalls on a paired
            # gather's completion
            Gs = [[gpool.tile([P, H], F32, tag="G", name=f"G_{r}_{c}")
                   for c in range(CHUNKS)] for r in range(ROWS)]
            for i in range(T0):
                for r in range(ROWS):
                    for c in range(CHUNKS):
                        j = (r * CHUNKS + c) * T0 + i
                        nc.gpsimd.indirect_dma_start(
                            out=Gs[r][c][:], out_offset=None, in_=embed[:],
                            in_offset=IndirectOffsetOnAxis(
                                ap=ids_sb[:, j:j + 1], axis=0),
                            compute_op=(AOP.bypass if i == 0 else AOP.add))
            for r in range(ROWS):
                # tgt dots first: the output broadcast needs ALL of them
                ccs = []
                for c in range(CHUNKS):
                    prod = spool.tile([P, H], F32, tag="prod")
                    nc.vector.tensor_tensor(out=prod[:], in0=Gs[r][c][:],
                                            in1=wtgt_sb[:], op=AOP.mult)
                    c_c = wpool.tile([P, 1], F32, tag="ccol",
                                     name=f"cc_{r}_{c}")
                    thr = spool.tile([P, H], F32, tag="thr")
                    nc.scalar.activation(out=thr[:], in_=prod[:], func=AF.Copy,
                                         accum_out=c_c[:, 0:1])
                    ccs.append(c_c)
                ct_sb = wpool.tile([P, S], F32, tag="ctsb")
                for c in range(CHUNKS):
                    ct_ps = psl.tile([P, P], F32, tag="ctps", space="PSUM")
                    nc.tensor.transpose(out=ct_ps[0:1, 0:P],
                                        in_=ccs[c][:, 0:1], identity=id_sb[:])
                    nc.vector.tensor_scalar(out=ct_sb[0:1, c * P:(c + 1) * P],
                                            in0=ct_ps[0:1, 0:P],
                                            scalar1=float(bias_val),
                                            scalar2=None, op0=AOP.add)
                bc_ps = psl.tile([P, S], F32, tag="bcps", space="PSUM")
                for c in range(CHUNKS):
                    nc.tensor.matmul(out=bc_ps[:, c * P:(c + 1) * P],
                                     lhsT=ones[0:1, 0:P],
                                     rhs=ct_sb[0:1, c * P:(c + 1) * P],
                                     start=True, stop=True)
                # src dots: each s-chunk's output row block ships as soon as
                # its own dot lands
                for sc in range(CHUNKS):
                    prod = spool.tile([P, H], F32, tag="prod")
                    nc.vector.tensor_tensor(out=prod[:], in0=Gs[r][sc][:],
                                            in1=wsrc_sb[:], op=AOP.mult)
                    a_c = wpool.tile([P, 1], F32, tag="acol",
                                     name=f"ac_{r}_{sc}")
                    thr = spool.tile([P, H], F32, tag="thr")
                    nc.scalar.activation(out=thr[:], in_=prod[:], func=AF.Copy,
                                         accum_out=a_c[:, 0:1])
                    o_sb = opool.tile([P, S], F32, tag="osb")
                    nc.vector.tensor_scalar(out=o_sb[:], in0=bc_ps[:],
                                            scalar1=a_c[:, 0:1], scalar2=None,
                                            op0=AOP.add)
                    nc.sync.dma_start(out=out[r, sc * P:(sc + 1) * P, :],
                                      in_=o_sb[:])
    nc.compile()
    return nc


def _build_general(sched_src, sched_tgt, same_wid, bias_val):
    """General sorted-word-ids kernel via one-hot PE segment-sum."""
    nc = bacc.Bacc("TRN2", target_bir_lowering=False, debug=False,
                   num_devices=N_CORES)
    embed = nc.dram_tensor("embed", [V, H], F32, kind="ExternalInput")
    ids = nc.dram_tensor("ids", [P, ROWS * TILES], mybir.dt.int32,
                         kind="ExternalInput")
    wids = nc.dram_tensor("wids", [P, ROWS * TILES], F32, kind="ExternalInput")
    if not same_wid:
        widt = nc.dram_tensor("widt", [P, ROWS * TILES], F32,
                              kind="ExternalInput")
    mask = nc.dram_tensor("mask", [P, ROWS * TILES], F32, kind="ExternalInput")
    wb = nc.dram_tensor("wb", [2, P, H], F32, kind="ExternalInput")
    iota = nc.dram_tensor("iota", [P, S], F32, kind="ExternalInput")
    ident = nc.dram_tensor("ident", [P, P], F32, kind="ExternalInput")
    out = nc.dram_tensor("out", [ROWS, S, S], F32, kind="ExternalOutput")

    with TileContext(nc) as tc:
        with (
            tc.tile_pool(name="const", bufs=1) as cpool,
            tc.tile_pool(name="hid", bufs=2 * TILES) as hpool,
            tc.tile_pool(name="work", bufs=4) as wpool,
            tc.tile_pool(name="scratch", bufs=2) as spool,
            tc.tile_pool(name="outp", bufs=4) as opool,
            tc.tile_pool(name="pg", bufs=2, space="PSUM") as pg,
            tc.tile_pool(name="psl", bufs=1, space="PSUM") as psl,
        ):
            ids_sb = cpool.tile([P, ROWS * TILES], mybir.dt.int32, tag="ids")
            nc.sync.dma_start(out=ids_sb[:], in_=ids[:])
            ws_sb = cpool.tile([P, ROWS * TILES], F32, tag="wids")
            nc.sync.dma_start(out=ws_sb[:], in_=wids[:])
            if not same_wid:
                wt_sb = cpool.tile([P, ROWS * TILES], F32, tag="widt")
                nc.sync.dma_start(out=wt_sb[:], in_=widt[:])
            mk_sb = cpool.tile([P, ROWS * TILES], F32, tag="mask")
            nc.sync.dma_start(out=mk_sb[:], in_=mask[:])
            wsrc_sb = cpool.tile([P, H], F32, tag="wsrc")
            nc.sync.dma_start(out=wsrc_sb[:], in_=wb[0])
            wtgt_sb = cpool.tile([P, H], F32, tag="wtgt")
            nc.sync.dma_start(out=wtgt_sb[:], in_=wb[1])
            iota_sb = cpool.tile([P, S], F32, tag="iota")
            nc.sync.dma_start(out=iota_sb[:], in_=iota[:])
            id_sb = cpool.tile([P, P], F32, tag="ident")
            nc.sync.dma_start(out=id_sb[:], in_=ident[:])
            ones = cpool.tile([P, P], F32, tag="ones")
            nc.vector.memset(ones[:], 1.0)

            for r in range(ROWS):
                hid = []
                for t in range(TILES):
                    h_t = hpool.tile([P, H], F32, tag="hid")
                    nc.gpsimd.indirect_dma_start(
                        out=h_t[:], out_offset=None, in_=embed[:],
                        in_offset=IndirectOffsetOnAxis(
                            ap=ids_sb[:, r * TILES + t: r * TILES + t + 1],
                            axis=0))
                    hid.append(h_t)

                acols = wpool.tile([P, CHUNKS], F32, tag="acols")
                ccols = wpool.tile([P, CHUNKS], F32, tag="ccols")

                def g_phase(wid_sb, sched, dots):
                    for c in range(CHUNKS):
                        G = pg.tile([P, 3 * 512], F32, tag="G")
                        tiles = sched[c] if sched[c] else [0]
                        n = len(tiles)
                        for j, t in enumerate(tiles):
                            oh = wpool.tile([P, P], F32, tag="oh")
                            col = slice(r * TILES + t, r * TILES + t + 1)
                            nc.vector.tensor_scalar(
                                out=oh[:], in0=iota_sb[:, c * P:(c + 1) * P],
                                scalar1=wid_sb[:, col], scalar2=mk_sb[:, col],
                                op0=AOP.is_equal, op1=AOP.mult)
                            st, sp = (j == 0), (j == n - 1)
                            nc.tensor.matmul(out=G[:, 0:512], lhsT=oh[:],
                                             rhs=hid[t][:, 0:512],
                                             start=st, stop=sp)
                            nc.tensor.matmul(out=G[:, 512:1024], lhsT=oh[:],
                                             rhs=hid[t][:, 512:1024],
                                             start=st, stop=sp)
                            nc.tensor.matmul(out=G[:, 1024:1025], lhsT=oh[:],
                                             rhs=ones[:, 0:1],
                                             start=st, stop=sp)
                        cnt = wpool.tile([P, 1], F32, tag="cnt")
                        nc.vector.tensor_scalar_max(out=cnt[:],
                                                    in0=G[:, 1024:1025],
                                                    scalar1=1.0)
                        rcnt = wpool.tile([P, 1], F32, tag="rcnt")
                        nc.vector.reciprocal(out=rcnt[:], in_=cnt[:])
                        for w_sb, cols in dots:
                            raw = wpool.tile([P, 1], F32, tag="raw")
                            prod = spool.tile([P, H], F32, tag="prod")
                            nc.vector.tensor_tensor(out=prod[:], in0=G[:, 0:H],
                                                    in1=w_sb[:], op=AOP.mult)
                            thr = spool.tile([P, H], F32, tag="thr")
                            nc.scalar.activation(out=thr[:], in_=prod[:],
                                                 func=AF.Copy,
                                                 accum_out=raw[:])
                            nc.vector.tensor_tensor(out=cols[:, c:c + 1],
                                                    in0=raw[:], in1=rcnt[:],
                                                    op=AOP.mult)

                if same_wid:
                    g_phase(ws_sb, sched_src[r],
                            [(wsrc_sb, acols), (wtgt_sb, ccols)])
                else:
                    g_phase(ws_sb, sched_src[r], [(wsrc_sb, acols)])
                    g_phase(wt_sb, sched_tgt[r], [(wtgt_sb, ccols)])
                _out_assembly(nc, wpool, psl, ones, id_sb, acols, ccols,
                              out, r, bias_val, opool)
    nc.compile()
    return nc


def _cols(x, dtype):
    """[ROWS, L] -> [P, ROWS*TILES]; column r*TILES+t row p = x[r, t*P+p]."""
    return np.ascontiguousarray(
        x.reshape(ROWS, TILES, P).transpose(2, 0, 1)
        .reshape(P, ROWS * TILES).astype(dtype))


def _cols_fast(x):
    """[ROWS, L] -> [P, ROWS*CHUNKS*T0]; col (r*CHUNKS+c)*T0+i row p
    = x[r, T0*(c*P + p) + i]."""
    # x.reshape(ROWS, CHUNKS, P, T0)[r, c, p, i] = x[r, (c*P+p)*T0 + i]
    xr = x.reshape(ROWS, CHUNKS, P, T0).transpose(2, 0, 1, 3)
    return np.ascontiguousarray(
        xr.reshape(P, ROWS * CHUNKS * T0).astype(np.int32))


def _mk_sched(wid, msk):
    """Union (over cores) of token tiles touching each word chunk."""
    sched = [[set() for _ in range(CHUNKS)] for _ in range(ROWS)]
    for row in range(B):
        r = row % ROWS
        wrow = wid[row]
        mrow = msk[row]
        for t in range(TILES):
            w = wrow[t * P:(t + 1) * P]
            m = mrow[t * P:(t + 1) * P]
            w = w[m > 0]
            if w.size == 0:
                continue
            lo = max(int(w.min()) // P, 0)
            hi = min(int(w.max()) // P, CHUNKS - 1)
            for c in range(lo, hi + 1):
                sched[r][c].add(t)
    return tuple(tuple(tuple(sorted(s)) for s in row) for row in sched)


_REG_WID = np.arange(L) // T0


def _is_regular(ws, wt, msk):
    return (np.all(msk == 1)
            and np.array_equal(ws, np.broadcast_to(_REG_WID, ws.shape))
            and np.array_equal(wt, np.broadcast_to(_REG_WID, wt.shape)))


def kernel(input_ids, attention_mask, source_word_ids, target_word_ids,
           embed, classifier_w, classifier_b, _trace=False):
    global LAST_EXEC_NS, LAST_RESULTS
    ids = np.asarray(input_ids).astype(np.int64)
    msk = np.asarray(attention_mask).astype(np.int64)
    ws = np.asarray(source_word_ids).astype(np.int64)
    wt = np.asarray(target_word_ids).astype(np.int64)
    emb = np.ascontiguousarray(np.asarray(embed, dtype=np.float32))
    w2 = np.asarray(classifier_w, dtype=np.float32).reshape(2, H)
    bias = float(np.asarray(classifier_b, dtype=np.float32).reshape(-1)[0])

    ident_np = np.eye(P, dtype=np.float32)
    fast = _is_regular(ws, wt, msk)

    if fast:
        import os
        variant = os.environ.get("KERNEL_FAST_VARIANT", "raw")
        key = ("fast", variant, bias)
        nc = _CACHE.get(key)
        if nc is None:
            builder = _build_fast_raw if variant == "raw" else _build_fast
            nc = _CACHE[key] = builder(bias)
        w2s = w2 / float(T0)        # fold the mean divisor into w (exact)
        wbc = np.ascontiguousarray(
            np.broadcast_to(w2s.reshape(2, 1, H), (2, P, H)))
        in_maps = []
        for k in range(N_CORES):
            rows = slice(k * ROWS, (k + 1) * ROWS)
            in_maps.append({
                "embed": emb,
                "ids": _cols_fast(ids[rows]),
                "wb": wbc,
                "ident": ident_np,
            })
    else:
        same_wid = np.array_equal(ws, wt)
        sched_src = _mk_sched(ws, msk)
        sched_tgt = sched_src if same_wid else _mk_sched(wt, msk)
        key = (same_wid, sched_src, sched_tgt, bias)
        nc = _CACHE.get(key)
        if nc is None:
            nc = _CACHE[key] = _build_general(sched_src, sched_tgt,
                                              same_wid, bias)
        wbc = np.ascontiguousarray(
            np.broadcast_to(w2.reshape(2, 1, H), (2, P, H)))
        iota_np = np.ascontiguousarray(
            np.tile(np.arange(S, dtype=np.float32), (P, 1)))
        in_maps = []
        for k in range(N_CORES):
            rows = slice(k * ROWS, (k + 1) * ROWS)
            m = {
                "embed": emb,
                "ids": _cols(ids[rows], np.int32),
                "wids": _cols(ws[rows], np.float32),
                "mask": _cols(msk[rows], np.float32),
                "wb": wbc,
                "iota": iota_np,
                "ident": ident_np,
            }
            if not same_wid:
                m["widt"] = _cols(wt[rows], np.float32)
            in_maps.append(m)

    res = bass_utils.run_bass_kernel_spmd(
        nc, in_maps, core_ids=list(range(N_CORES)), trace=_trace)
    LAST_EXEC_NS = res.exec_time_ns
    LAST_RESULTS = res
    return np.concatenate([res.results[k]["out"] for k in range(N_CORES)],
                          axis=0)

